# revision 1
# baseline (speedup 1.0000x reference)
"""Trainium2 Bass kernel for CausalAttentionSortNet bucket-scoring.

Math (see reference): only `k` feeds the output. For each merged batch*head
slice, the cumulative-average of k is sampled at bucket starts (every 128th
row), which reduces to per-chunk sums + a strictly-triangular prefix matmul.
The rest is tiny per-bucket sort projections and a 64x65 masked softmax.

Sharding: data-parallel over the merged (batch*heads)=32 axis across 8 cores,
4 slices per core, processed as 2 pairs of 2 slices; a pair fills the
128-partition dim as partition=(slice_in_pair, chunk), free=(row, dim) so
every partition's k data is one contiguous 32KB HBM run (measured 381 GB/s).

`q` (half of all input bytes) is never read by the reference computation, so
it is not even transferred to the device.

Per 1MB sub-tile: GpSimd does the contiguous half-fold (rows r -> r/2),
VectorE the remaining strided reduce, and the PE accumulates the partial
chunk-sums of BOTH pairs at once into the transposed scaled prefix via
PT[(p,d),(b,j)] += par_both.T @ (s_j * tril-ones), with the partial as the
stationary operand -- no transpose, scale, or partial-fold instructions.
"""

from contextlib import ExitStack

import numpy as np

import concourse.bacc as bacc
import concourse.mybir as mybir
import concourse.tile as tile
from concourse import bass_utils

# Problem constants (hardcoded per contract; kernel.py must be self-contained).
B, HEADS, BUCKETS, DIM, DIM_SORT, T = 4, 8, 64, 64, 8, 8192
BH = B * HEADS            # 32 merged batch*head slices
NCORES = 8
BHC = BH // NCORES        # 4 slices per core
NPAIR = BHC // 2          # 2 pairs per core
CHUNK = T // BUCKETS      # 128 rows per bucket
NEG = -1.0e30             # softmax mask value (underflows exp to exactly 0)
FP = mybir.dt.float32

# rows-per-sub-tile; ascending stream order (first tiles small so compute
# starts early, biggest tile mid-stream, smaller one last to trim the tail
# chain; all even so the gp half-fold covers every row)
ROWS = (12, 12, 34, 36, 34)

TRACE = False  # set by test.py for profiling runs
TRACE_KWARGS = {}  # extra run_bass_kernel_spmd kwargs for profiling runs
LAST_RESULTS = None  # BassKernelResults of the most recent run

_PROG_CACHE = {}


def _build_program(t_seq=T, rows=ROWS, enable_asserts=False, debug_taps=False):
    chunk = t_seq // BUCKETS
    # scale sub-tile rows for reduced-T dev runs
    if chunk != CHUNK:
        base = [max(1, r * chunk // CHUNK) for r in rows]
        base[-1] = chunk - sum(base[:-1])
        rows = tuple(base)
    assert sum(rows) == chunk, (rows, chunk)

    nc = bacc.Bacc(
        "TRN2",
        target_bir_lowering=False,
        debug=False,
        enable_asserts=enable_asserts,
        num_devices=NCORES,
    )

    def din(name, shape):
        return nc.dram_tensor(name, shape, FP, kind="ExternalInput").ap()

    kin = din("kin", (BHC, t_seq, DIM))
    # packed constants (one DMA each):
    # c128: [lmat_s | idents | ident | amask_b | mmask_b | mask0_b]
    c128 = din("c128", (128, 128 * 5 + 2))
    # c64: [wqk_pt_p0 | wqk_pt_p1 | wqk_ft_p0 | wqk_ft_p1], each (·, 104):
    #   cols 0:40 = sort-q weights, cols 64:104 = sort-k weights; the 64 weight
    #   rows are duplicated into both partition halves so each pair's matmul
    #   can read them at the same base partition as its PT/FT slice
    c64 = din("c64", (128, 4 * 104))
    # c104: per pair a (104, 128) block: rows 0:40 = cqt, 64:104 = ckt — added
    # into the SKQ psum group by an identity-weighted matmul
    c104 = din("c104", (104, 2 * 128))
    rout = nc.dram_tensor(
        "rout", (BHC, BUCKETS, BUCKETS + 1), FP, kind="ExternalOutput"
    ).ap()
    taps = {}
    if debug_taps:
        taps["pt"] = nc.dram_tensor("tap_pt", (128, 128), FP, kind="ExternalOutput").ap()
        taps["ft"] = nc.dram_tensor("tap_ft", (128, 128), FP, kind="ExternalOutput").ap()
        taps["sq"] = nc.dram_tensor("tap_sq", (40, 2, 128), FP, kind="ExternalOutput").ap()
        taps["rk"] = nc.dram_tensor("tap_rk", (40, 2, 128), FP, kind="ExternalOutput").ap()

    X = mybir.AxisListType.X
    Exp = mybir.ActivationFunctionType.Exp
    MULT = mybir.AluOpType.mult
    nsub = len(rows)

    with tile.TileContext(nc) as tc:
        with ExitStack() as ctx:
            singles = ctx.enter_context(tc.tile_pool(name="singles", bufs=1))
            kpool = ctx.enter_context(tc.tile_pool(name="kpool", bufs=2 * nsub))
            parts = ctx.enter_context(tc.tile_pool(name="parts", bufs=nsub))
            small = ctx.enter_context(tc.tile_pool(name="small", bufs=2))
            pp = ctx.enter_context(tc.tile_pool(name="pp", bufs=1, space="PSUM"))

            # ---- first row of every chunk for both pairs, one tiny DMA on the
            # scalar queue (its many small descriptors must not delay the bulk
            # stream on the sync queue)
            ksrc4 = kin.rearrange(
                "(p b) (c r) d -> (b c) r p d", p=NPAIR, r=chunk
            )
            F_both = singles.tile([128, NPAIR, DIM], FP, tag="F")
            nc.scalar.dma_start(F_both[:], ksrc4[:, 0, :, :])

            # constants go out before pair-1's bulk tiles claim the scalar queue
            c128_sb = singles.tile([128, 128 * 5 + 2], FP, tag="c128")
            nc.scalar.dma_start(c128_sb[:], c128)
            c64_sb = singles.tile([128, 4 * 104], FP, tag="c64")
            nc.scalar.dma_start(c64_sb[:], c64)
            c104_sb = singles.tile([104, 2 * 128], FP, tag="c104")
            nc.scalar.dma_start(c104_sb[:], c104)

            # ---- bulk k sub-tile DMAs, alternating queues per pair
            ksrcs = [
                kin[2 * p : 2 * p + 2].rearrange("b (c r) d -> (b c) r d", r=chunk)
                for p in range(NPAIR)
            ]
            # one bulk queue: two-queue packet round-robin measured ~3.4 us
            # slower (per-packet queue switching beats any issue overlap)
            kts = {}
            r0 = 0
            for s, rs in enumerate(rows):
                for p in range(NPAIR):
                    kt = kpool.tile([128, rs, DIM], FP, tag="kt")
                    nc.sync.dma_start(kt[:], ksrcs[p][:, r0 : r0 + rs, :])
                    kts[(p, s)] = kt
                r0 += rs

            lmat_s = c128_sb[:, 0:128]
            idents = c128_sb[:, 128:256]
            ident = c128_sb[:, 256:384]
            # amask (cols 384:512) is consumed directly by the R-group matmul
            mmask_b = c128_sb[:, 512:640].rearrange("q (p j) -> q p j", p=2)
            mask0_b = c128_sb[:, 640:642]

            # ---- per-sub-tile reduction: gp half-fold + DVE strided reduce,
            # PE accumulates both pairs at once (one clean group per bank)
            PT_ps = pp.tile([128, 128], FP, tag="PT")
            FT_ps = pp.tile([128, 128], FP, tag="FT")
            nc.tensor.matmul(
                PT_ps[:], lhsT=F_both[:], rhs=idents, start=True, stop=False
            )
            nc.tensor.matmul(
                FT_ps[:], lhsT=F_both[:], rhs=ident, start=True, stop=True
            )
            for s, rs in enumerate(rows):
                par = parts.tile([128, NPAIR, DIM], FP, tag="par")
                for p in range(NPAIR):
                    kt = kts[(p, s)]
                    h = rs
                    if rs > 8:
                        h = rs // 2
                        if s == nsub - 1 and p == NPAIR - 1:
                            # very last tile: fold on DVE (GpSimd is still
                            # draining its previous fold by then, and the
                            # cross-engine handoff would sit on the tail)
                            nc.vector.tensor_add(
                                kt[:, 0:h, :], kt[:, 0:h, :], kt[:, h : 2 * h, :]
                            )
                        else:
                            nc.gpsimd.tensor_add(
                                kt[:, 0:h, :], kt[:, 0:h, :], kt[:, h : 2 * h, :]
                            )
                    nc.vector.reduce_sum(
                        par[:, p, :],
                        kt[:, 0:h, :].rearrange("p r d -> p d r"),
                        axis=X,
                    )
                nc.tensor.matmul(
                    PT_ps[:],
                    lhsT=par[:],
                    rhs=lmat_s,
                    start=False,
                    stop=s == nsub - 1,
                )

            # ---- sort projections (per pair), batched softmax (both pairs)
            PT_sb = small.tile([128, 128], FP, tag="PTs")
            nc.scalar.copy(PT_sb[:], PT_ps[:])
            FT_sb = small.tile([128, 128], FP, tag="FTs")
            nc.scalar.copy(FT_sb[:], FT_ps[:])
            if debug_taps:
                nc.sync.dma_start(taps["pt"], PT_sb[:])
                nc.sync.dma_start(taps["ft"], FT_sb[:])

            # SKQ rows: 0:40 sort-q blocks (b0 at 0:8, b1 at 32:40),
            #           64:104 sort-k blocks (b0 at 64:72, b1 at 96:104);
            # one PSUM bank per pair: each holds a long-open accumulation group
            # opened by the constant-term matmul (ready at kernel start) and
            # closed by the PT-part matmul (the only one on the critical tail)
            SQs = []
            RKs = []
            for p in range(NPAIR):
                sk_ps_t = pp.tile([104, 128], FP, tag=f"SKQ{p}")
                sk_ps = sk_ps_t[:]
                prow = slice(64 * p, 64 * p + 64)
                nc.tensor.matmul(
                    sk_ps,
                    lhsT=ident[0:104, 0:104],
                    rhs=c104_sb[:, 128 * p : 128 * p + 128],
                    start=True,
                    stop=False,
                    skip_group_check=True,
                )
                nc.tensor.matmul(
                    sk_ps,
                    lhsT=c64_sb[prow, 208 + 104 * p : 208 + 104 * p + 104],
                    rhs=FT_sb[prow, :],
                    start=False,
                    stop=False,
                    skip_group_check=True,
                )
                nc.tensor.matmul(
                    sk_ps,
                    lhsT=c64_sb[prow, 104 * p : 104 * p + 104],
                    rhs=PT_sb[prow, :],
                    start=False,
                    stop=True,
                    skip_group_check=True,
                )
                sq_sb = small.tile([40, 128], FP, tag=f"SQ{p}")
                nc.scalar.copy(sq_sb[:], sk_ps[0:40, :])
                rk_sb = small.tile([40, 128], FP, tag=f"RK{p}")
                nc.vector.tensor_copy(rk_sb[:], sk_ps[64:104, :])
                SQs.append(sq_sb)
                RKs.append(rk_sb)
            if debug_taps:
                for p in range(NPAIR):
                    nc.sync.dma_start(taps["sq"][:, p, :], SQs[p][:])
                    nc.sync.dma_start(taps["rk"][:, p, :], RKs[p][:])

            # R group: opened early by an identity-weighted matmul that seeds
            # the bank with the additive causal mask; the four sq.sk matmuls
            # then accumulate into their quadrants, so the masked logits sit
            # in PSUM with no extra elementwise pass
            R_ps = pp.tile([128, 128], FP, tag="R")
            nc.tensor.matmul(
                R_ps[:],
                lhsT=ident,
                rhs=c128_sb[:, 384:512],
                start=True,
                stop=False,
                skip_group_check=True,
            )
            for p in range(NPAIR):
                nc.tensor.matmul(
                    R_ps[0:64, 64 * p : 64 * p + 64],
                    lhsT=SQs[p][0:8, 0:64],
                    rhs=RKs[p][0:8, 0:64],
                    start=False,
                    stop=False,
                    skip_group_check=True,
                )
                nc.tensor.matmul(
                    R_ps[64:128, 64 * p : 64 * p + 64],
                    lhsT=SQs[p][32:40, 64:128],
                    rhs=RKs[p][32:40, 64:128],
                    start=False,
                    stop=p == NPAIR - 1,
                    skip_group_check=True,
                )

            # masked softmax over 65 logits (implicit zero-logit column 0),
            # both pairs batched along the free axis: cols = (pair, j)
            Rm = R_ps[:].rearrange("q (p j) -> q p j", p=2)
            mx = small.tile([128, 2], FP, tag="mx")
            nc.vector.reduce_max(mx[:], Rm, axis=X)
            negm = small.tile([128, 2], FP, tag="negm")
            nc.vector.tensor_scalar(
                negm[:], mx[:], 0.0, -1.0,
                op0=mybir.AluOpType.max, op1=MULT,
            )
            e0 = small.tile([128, 2], FP, tag="e0")
            nc.scalar.activation(e0[:], negm[:], Exp)
            e_sb = small.tile([128, 2, 64], FP, tag="e")
            for p in range(NPAIR):
                nc.scalar.activation(
                    e_sb[:, p, :], R_ps[:, 64 * p : 64 * p + 64], Exp,
                    bias=negm[:, p : p + 1], scale=1.0,
                )
            s1 = small.tile([128, 2], FP, tag="s1")
            nc.vector.reduce_sum(s1[:], e_sb[:], axis=X)
            den = small.tile([128, 2], FP, tag="den")
            nc.vector.tensor_add(den[:], s1[:], e0[:])
            rin = small.tile([128, 2], FP, tag="rin")
            nc.vector.reciprocal(rin[:], den[:])
            outt = small.tile([128, 2, BUCKETS + 1], FP, tag="outt")
            for p in range(NPAIR):
                # outt = (e * 1/den) * tril-mask, fused
                nc.vector.scalar_tensor_tensor(
                    outt[:, p, 1:],
                    e_sb[:, p, :],
                    rin[:, p : p + 1],
                    mmask_b[:, p, :],
                    op0=MULT,
                    op1=MULT,
                )
            t0 = small.tile([128, 2], FP, tag="t0")
            nc.vector.tensor_mul(t0[:], e0[:], mask0_b)
            nc.vector.tensor_mul(outt[:, :, 0], t0[:], rin[:])
            nc.sync.dma_start(
                rout.rearrange("(p b) i c -> (b i) p c", p=2), outt[:]
            )

    nc.compile()
    return nc


def _get_program(t_seq=T, enable_asserts=False):
    key = (t_seq, enable_asserts)
    if key not in _PROG_CACHE:
        _PROG_CACHE[key] = _build_program(t_seq, enable_asserts=enable_asserts)
    return _PROG_CACHE[key]


def _host_constants(core, q_pos_emb, k_pos_emb, Wsq, Wsk, chunk=CHUNK):
    """Tiny per-core packed constant tensors."""
    f32 = np.float32
    j = np.arange(64, dtype=np.float64)
    s = (1.0 / (chunk * j + 1.0)).astype(f32)  # per-bucket cumavg scale

    tri = np.triu(np.ones((64, 64), f32), k=1)  # [c, j] = 1 iff c < j
    tri_s = tri * s[None, :]
    lmat_s = np.zeros((128, 128), f32)
    lmat_s[0:64, 0:64] = tri_s
    lmat_s[64:128, 64:128] = tri_s
    idents = np.zeros((128, 128), f32)
    idents[np.arange(128), np.arange(128)] = np.concatenate([s, s])
    ident = np.eye(128, dtype=f32)

    rows = np.arange(64)[:, None]
    cols = np.arange(64)[None, :]
    am = np.where(cols < rows, 0.0, NEG).astype(f32)       # softmax additive mask
    mm = (cols <= rows - 2).astype(f32)                    # output tril(-1) mask
    amask_b = np.concatenate([am, am], axis=1)
    amask_b = np.concatenate([amask_b, amask_b], axis=0)
    mmask_b = np.concatenate([mm, mm], axis=1)
    mmask_b = np.concatenate([mmask_b, mmask_b], axis=0)
    m0 = (np.arange(64) > 0).astype(f32).reshape(64, 1)
    mask0_b = np.concatenate([np.concatenate([m0, m0], 1)] * 2, 0)

    c128 = np.concatenate([lmat_s, idents, ident, amask_b, mmask_b, mask0_b], axis=1)

    wq_pt = np.zeros((2, 64, 104), f32)   # [pair][d][sq 0:40 | sk 64:104]
    wq_ft = np.zeros((2, 64, 104), f32)
    cblk = np.zeros((2, 104, 128), f32)   # [pair][skq-row][(b, j)]
    for p in range(NPAIR):
        for b in range(2):
            bh = core * BHC + 2 * p + b
            h = bh % HEADS
            r0 = 32 * b
            wq_pt[p, :, r0 : r0 + 8] = Wsq[0, h, 0:64, :]
            wq_pt[p, :, 64 + r0 : 64 + r0 + 8] = Wsk[0, h, 0:64, :]
            wq_ft[p, :, r0 : r0 + 8] = Wsq[0, h, 64:128, :]
            wq_ft[p, :, 64 + r0 : 64 + r0 + 8] = Wsk[0, h, 64:128, :]
            cq = q_pos_emb[0, h] @ Wsq[0, h, 128:192, :]  # (64, 8)
            ck = k_pos_emb[0, h] @ Wsk[0, h, 128:192, :]
            cblk[p, r0 : r0 + 8, 64 * b : 64 * b + 64] = cq.T
            cblk[p, 64 + r0 : 64 + r0 + 8, 64 * b : 64 * b + 64] = ck.T

    c64 = np.concatenate([wq_pt[0], wq_pt[1], wq_ft[0], wq_ft[1]], axis=1)
    c64 = np.concatenate([c64, c64], axis=0)  # duplicate into both halves
    c104 = np.concatenate([cblk[0], cblk[1]], axis=1)
    return {"c128": c128, "c64": c64, "c104": c104}


def _run(k, q_pos_emb, k_pos_emb, Wsq, Wsk, trace=False, t_seq=T):
    nc = _get_program(t_seq)
    in_maps = []
    for core in range(NCORES):
        cm = _host_constants(
            core, q_pos_emb, k_pos_emb, Wsq, Wsk, chunk=t_seq // BUCKETS
        )
        cm["kin"] = np.ascontiguousarray(k[core * BHC : (core + 1) * BHC])
        in_maps.append(cm)
    res = bass_utils.run_bass_kernel_spmd(
        nc,
        in_maps,
        core_ids=list(range(NCORES)),
        trace=trace,
        **(TRACE_KWARGS if trace else {}),
    )
    global LAST_RESULTS
    LAST_RESULTS = res
    out = np.concatenate([r["rout"] for r in res.results], axis=0)
    return out, res


def kernel(**inputs):
    k = np.asarray(inputs["k"], np.float32)
    q_pos_emb = np.asarray(inputs["q_pos_emb"], np.float32)
    k_pos_emb = np.asarray(inputs["k_pos_emb"], np.float32)
    Wsq = np.asarray(inputs["Wsq"], np.float32)
    Wsk = np.asarray(inputs["Wsk"], np.float32)
    out, _ = _run(k, q_pos_emb, k_pos_emb, Wsq, Wsk, trace=TRACE)
    return out



# revision 10
# speedup vs baseline: 1.0948x; 1.0948x over previous
"""Trainium2 Bass kernel for CausalAttentionSortNet bucket-scoring.

Math (see reference): only `k` feeds the output. For each merged batch*head
slice, the cumulative-average of k is sampled at bucket starts (every 128th
row), which reduces to per-chunk sums + a strictly-triangular prefix matmul.
The rest is tiny per-bucket sort projections and a 64x65 masked softmax.

Sharding: data-parallel over the merged (batch*heads)=32 axis across 8 cores,
4 slices per core as 2 pairs; partition=(slice_in_pair, chunk), free=(row, dim)
so every partition's k data is one contiguous 32KB HBM run. Both pairs of each
row-group share one SBUF tile so each fold is a single batched instruction.

`q` (half of all input bytes) is never read by the reference computation, so
it is not even transferred to the device.

Per row-group sub-tile: contiguous pairwise fold chains on DVE (unit-stride
fp32 tensor_tensor), with GpSimd taking a d-column share of the first level
in parallel. The folded row 0 feeds the PE prefix matmul (PT). F (row 0 of
chunk) is read straight out of sub-tile 0 before the folds clobber it - no
separate gather DMA. The 64x65 softmax keeps the zero-logit column explicit
in PSUM so the tail is one exp + one scale-mask per pair. Junk matmuls gated
on the stream keep the PE's HAM clock at 8/8 for the tail matmuls.
"""

from contextlib import ExitStack

import numpy as np

import concourse.bacc as bacc
import concourse.mybir as mybir
import concourse.tile as tile
from concourse import bass_utils

# Problem constants (hardcoded per contract; kernel.py must be self-contained).
B, HEADS, BUCKETS, DIM, DIM_SORT, T = 4, 8, 64, 64, 8, 8192
BH = B * HEADS            # 32 merged batch*head slices
NCORES = 8
BHC = BH // NCORES        # 4 slices per core
NPAIR = BHC // 2          # 2 pairs per core
CHUNK = T // BUCKETS      # 128 rows per bucket
NEG = -1.0e30             # softmax mask value (underflows exp to exactly 0)
FP = mybir.dt.float32
BF = mybir.dt.bfloat16

# rows-per-sub-tile (per pair); ascending-then-descending so folds start early
# and the tail tile is tiny. Sum = 128.
ROWS = (12, 24, 32, 32, 16, 8, 4)
# sub-tiles whose first fold level is d-column-split with GpSimd (cols 32:64)
GP_L1 = (0, 1, 2, 3, 4)
# sub-tile whose *second* level goes fully to GpSimd
GP_L2 = (2,)
DCOL = 32  # GpSimd takes d columns [DCOL:64) of level-1

TRACE = False  # set by test.py for profiling runs
TRACE_KWARGS = {}  # extra run_bass_kernel_spmd kwargs for profiling runs
LAST_RESULTS = None  # BassKernelResults of the most recent run

_PROG_CACHE = {}


def _chain(rs):
    """Pairwise fold schedule for rs rows.

    Returns (ops, final) where ops are in-place (dst_lo, dst_hi, src_lo,
    src_hi) folds and final = (row_a, row_b) whose sum is the column total
    (written to the separate contiguous par tile so it can be a matmul lhsT).
    """
    ops = []
    leftovers = []
    n = rs
    while n > 3:
        h = n // 2
        ops.append((0, h, h, 2 * h))
        if n % 2:
            leftovers.append(n - 1)
        n = h
    if n == 3:
        ops.append((1, 2, 2, 3))
        n = 2
    assert n == 2, rs
    for r in leftovers:
        ops.append((1, 2, r, r + 1))
    return ops, (0, 1)


def _build_program(enable_asserts=False):
    assert sum(ROWS) == CHUNK, (ROWS, CHUNK)
    nsub = len(ROWS)

    nc = bacc.Bacc(
        "TRN2",
        target_bir_lowering=False,
        debug=False,
        enable_asserts=enable_asserts,
        num_devices=NCORES,
    )

    def din(name, shape, dt=FP):
        return nc.dram_tensor(name, shape, dt, kind="ExternalInput").ap()

    kin = din("kin", (BHC, T, DIM))
    # packed fp32 constants (one DMA):
    # c128: [lmat_s | idents | ident | amask65 | mmask65]
    c128 = din("c128", (128, 128 * 3 + 2 * (BUCKETS + 1)))
    # cb: bf16 identity for PE warm-up matmuls
    cb = din("cb", (128, 128), BF)
    # c64: [wqk_pt_p0 | wqk_pt_p1 | wqk_ft_p0 | wqk_ft_p1], each (., 104):
    #   cols 0:40 = sort-q weights, cols 64:104 = sort-k weights; the 64 weight
    #   rows are duplicated into both partition halves so each pair's matmul
    #   can read them at the same base partition as its PT/FT slice
    c64 = din("c64", (128, 4 * 104))
    # c104: per pair a (104, 128) block: rows 0:40 = cqt, 64:104 = ckt - added
    # into the SKQ psum group by an identity-weighted matmul
    c104 = din("c104", (104, 2 * 128))
    # out layout (b, i, pair, col): 520B contiguous per (b, i) partition
    rout = nc.dram_tensor(
        "rout", (2, BUCKETS, NPAIR, BUCKETS + 1), FP, kind="ExternalOutput"
    ).ap()

    Exp = mybir.ActivationFunctionType.Exp
    MULT = mybir.AluOpType.mult
    X = mybir.AxisListType.X

    with tile.TileContext(nc) as tc:
        with ExitStack() as ctx:
            singles = ctx.enter_context(tc.tile_pool(name="singles", bufs=1))
            kpool = ctx.enter_context(tc.tile_pool(name="kpool", bufs=1))
            small = ctx.enter_context(tc.tile_pool(name="small", bufs=2))
            pp = ctx.enter_context(tc.tile_pool(name="pp", bufs=1, space="PSUM"))

            # ---- constant DMAs on the scalar queue (bulk owns sync)
            c128_sb = singles.tile([128, 128 * 3 + 2 * (BUCKETS + 1)], FP, tag="c128")
            nc.scalar.dma_start(c128_sb[:], c128)
            cb_sb = singles.tile([128, 128], BF, tag="cb")
            nc.scalar.dma_start(cb_sb[:], cb)
            c64_sb = singles.tile([128, 4 * 104], FP, tag="c64")
            nc.scalar.dma_start(c64_sb[:], c64)
            c104_sb = singles.tile([104, 2 * 128], FP, tag="c104")
            nc.scalar.dma_start(c104_sb[:], c104)

            # ---- bulk k DMAs on the sync queue, pairs interleaved into one
            # shared tile per sub-tile so folds batch both pairs
            ksrcs = [
                kin[2 * p : 2 * p + 2].rearrange("b (c r) d -> (b c) r d", r=CHUNK)
                for p in range(NPAIR)
            ]
            kts = []
            r0 = 0
            for s, rs in enumerate(ROWS):
                kt = kpool.tile([128, NPAIR, rs, DIM], FP, tag=f"kt{s}")
                for p in range(NPAIR):
                    nc.sync.dma_start(kt[:, p], ksrcs[p][:, r0 : r0 + rs, :])
                kts.append(kt)
                r0 += rs

            lmat_s = c128_sb[:, 0:128]
            idents = c128_sb[:, 128:256]
            ident = c128_sb[:, 256:384]
            amask = c128_sb[:, 384 : 384 + 65]
            mmask = c128_sb[:, 449 : 449 + 65]

            # ---- PSUM tiles
            PT_ps = pp.tile([128, 128], FP, tag="PT")
            FT_ps = pp.tile([128, 128], FP, tag="FT")
            SKQs = [
                pp.tile([104, 128], FP, tag=f"SKQ{p}", name=f"SKQ{p}")
                for p in range(NPAIR)
            ]
            R_ps = pp.tile([128, NPAIR, BUCKETS + 1], FP, tag="R")
            dummy = pp.tile([128, 128], FP, tag="dummy")

            # ---- early PE work (consts-gated): R mask seeds + SKQ const seeds
            # one start=True per PSUM bank: start clears has_written bank-wide,
            # so only the first seed may carry it
            for p in range(NPAIR):
                nc.tensor.matmul(
                    R_ps[:, p, :], lhsT=ident, rhs=amask,
                    start=p == 0, stop=False, skip_group_check=True,
                )
            for p in range(NPAIR):
                nc.tensor.matmul(
                    SKQs[p][:], lhsT=ident[0:104, 0:104],
                    rhs=c104_sb[:, 128 * p : 128 * p + 128],
                    start=True, stop=False, skip_group_check=True,
                )

            # ---- F path: copy row 0 of sub-tile 0 to a contiguous tile
            # (matmul lhsT needs one flat free dim) before folds clobber it
            kt0 = kts[0]
            F_sb = small.tile([128, NPAIR, DIM], FP, tag="F")
            nc.vector.tensor_copy(F_sb[:], kt0[:, :, 0, :])
            nc.tensor.matmul(
                PT_ps[:], lhsT=F_sb[:], rhs=idents,
                start=True, stop=False, skip_group_check=True,
            )
            nc.tensor.matmul(
                FT_ps[:], lhsT=F_sb[:], rhs=ident, start=True, stop=True
            )
            FT_sb = small.tile([128, 128], FP, tag="FTs")
            nc.scalar.copy(FT_sb[:], FT_ps[:])
            for p in range(NPAIR):
                prow = slice(64 * p, 64 * p + 64)
                nc.tensor.matmul(
                    SKQs[p][:],
                    lhsT=c64_sb[prow, 208 + 104 * p : 208 + 104 * p + 104],
                    rhs=FT_sb[prow, :],
                    start=False, stop=False, skip_group_check=True,
                )

            # ---- per-sub-tile: batched fold chain then PT accumulation.
            # Junk matmuls gated on each pair-DMA keep the PE HAM clock warm.
            for s, rs in enumerate(ROWS):
                kt = kts[s]
                if s >= 1 and s < nsub - 1:
                    for p in range(NPAIR):
                        nc.tensor.matmul(
                            dummy[0:64, :], lhsT=kt[:, p, rs - 1, :], rhs=ident,
                            start=True, stop=True, skip_group_check=True,
                        )
                ops, (fa, fb) = _chain(rs)
                for i, (dlo, dhi, slo, shi) in enumerate(ops):
                    if i == 0 and s in GP_L1:
                        nc.vector.tensor_add(
                            kt[:, :, dlo:dhi, 0:DCOL],
                            kt[:, :, dlo:dhi, 0:DCOL],
                            kt[:, :, slo:shi, 0:DCOL],
                        )
                        nc.gpsimd.tensor_add(
                            kt[:, :, dlo:dhi, DCOL:DIM],
                            kt[:, :, dlo:dhi, DCOL:DIM],
                            kt[:, :, slo:shi, DCOL:DIM],
                        )
                    elif i == 1 and s in GP_L2:
                        nc.gpsimd.tensor_add(
                            kt[:, :, dlo:dhi, :],
                            kt[:, :, dlo:dhi, :],
                            kt[:, :, slo:shi, :],
                        )
                    else:
                        nc.vector.tensor_add(
                            kt[:, :, dlo:dhi, :],
                            kt[:, :, dlo:dhi, :],
                            kt[:, :, slo:shi, :],
                        )
                par = kpool.tile([128, NPAIR, DIM], FP, tag=f"par{s}", name=f"par{s}")
                nc.vector.tensor_add(par[:], kt[:, :, fa, :], kt[:, :, fb, :])
                nc.tensor.matmul(
                    PT_ps[:], lhsT=par[:], rhs=lmat_s,
                    start=False, stop=s == nsub - 1, skip_group_check=True,
                )
                if s == 0:
                    # warm-up burst: sustained PE activity flips HAM to 8/8
                    for _ in range(16):
                        nc.tensor.matmul(
                            dummy[:], lhsT=cb_sb[:], rhs=cb_sb[:],
                            start=True, stop=True, skip_group_check=True,
                        )

            # ---- tail: PT -> SBUF -> sort projections -> R -> softmax -> out
            PT_sb = small.tile([128, 128], FP, tag="PTs")
            nc.scalar.copy(PT_sb[:], PT_ps[:])
            for p in range(NPAIR):
                prow = slice(64 * p, 64 * p + 64)
                nc.tensor.matmul(
                    SKQs[p][:],
                    lhsT=c64_sb[prow, 104 * p : 104 * p + 104],
                    rhs=PT_sb[prow, :],
                    start=False, stop=True, skip_group_check=True,
                )
            SQs = []
            RKs = []
            for p in range(NPAIR):
                sq_sb = small.tile([40, 128], FP, tag=f"SQ{p}")
                nc.scalar.copy(sq_sb[:], SKQs[p][0:40, :])
                rk_sb = small.tile([40, 128], FP, tag=f"RK{p}")
                nc.vector.tensor_copy(rk_sb[:], SKQs[p][64:104, :])
                SQs.append(sq_sb)
                RKs.append(rk_sb)
            for p in range(NPAIR):
                nc.tensor.matmul(
                    R_ps[0:64, p, 1:],
                    lhsT=SQs[p][0:8, 0:64],
                    rhs=RKs[p][0:8, 0:64],
                    start=False, stop=False, skip_group_check=True,
                )
                nc.tensor.matmul(
                    R_ps[64:128, p, 1:],
                    lhsT=SQs[p][32:40, 64:128],
                    rhs=RKs[p][32:40, 64:128],
                    start=False, stop=p == NPAIR - 1, skip_group_check=True,
                )

            # masked softmax over 65 logits (zero-logit col 0 is explicit in
            # PSUM from the seed), both pairs batched where bias rules allow
            mx = small.tile([128, NPAIR], FP, tag="mx")
            nc.vector.reduce_max(mx[:], R_ps[:], axis=X)
            negm = small.tile([128, NPAIR], FP, tag="negm")
            nc.scalar.mul(negm[:], mx[:], -1.0)
            e_sb = small.tile([128, NPAIR, BUCKETS + 1], FP, tag="e")
            for p in range(NPAIR):
                nc.scalar.activation(
                    e_sb[:, p, :], R_ps[:, p, :], Exp,
                    bias=negm[:, p : p + 1], scale=1.0,
                )
            s1 = small.tile([128, NPAIR], FP, tag="s1")
            nc.vector.reduce_sum(s1[:], e_sb[:], axis=X)
            rin = small.tile([128, NPAIR], FP, tag="rin")
            nc.vector.reciprocal(rin[:], s1[:])
            outt = small.tile([128, NPAIR, BUCKETS + 1], FP, tag="outt")
            for p in range(NPAIR):
                # outt = (e * 1/den) * tril-mask, fused
                nc.vector.scalar_tensor_tensor(
                    outt[:, p, :],
                    e_sb[:, p, :],
                    rin[:, p : p + 1],
                    mmask,
                    op0=MULT,
                    op1=MULT,
                )
            nc.sync.dma_start(rout.rearrange("b i p c -> (b i) p c"), outt[:])

    nc.compile()
    return nc


def _get_program(enable_asserts=False):
    key = enable_asserts
    if key not in _PROG_CACHE:
        _PROG_CACHE[key] = _build_program(enable_asserts=enable_asserts)
    return _PROG_CACHE[key]


def _host_constants(core, q_pos_emb, k_pos_emb, Wsq, Wsk):
    """Tiny per-core packed constant tensors."""
    f32 = np.float32
    j = np.arange(64, dtype=np.float64)
    s = (1.0 / (CHUNK * j + 1.0)).astype(f32)  # per-bucket cumavg scale

    tri = np.triu(np.ones((64, 64), f32), k=1)  # [c, j] = 1 iff c < j
    tri_s = tri * s[None, :]
    lmat_s = np.zeros((128, 128), f32)
    lmat_s[0:64, 0:64] = tri_s
    lmat_s[64:128, 64:128] = tri_s
    idents = np.zeros((128, 128), f32)
    idents[np.arange(128), np.arange(128)] = np.concatenate([s, s])
    ident = np.eye(128, dtype=f32)

    q = np.arange(64)[:, None]
    jc = np.arange(65)[None, :]
    am = np.where(jc > q, NEG, 0.0).astype(f32)   # softmax additive mask, col0 free
    mm = (jc < q).astype(f32)                     # output tril(-1) mask incl col0
    amask = np.concatenate([am, am], axis=0)      # (128, 65) both b blocks
    mmask = np.concatenate([mm, mm], axis=0)

    c128 = np.concatenate([lmat_s, idents, ident, amask, mmask], axis=1)

    import ml_dtypes

    cb16 = np.eye(128, dtype=ml_dtypes.bfloat16)

    wq_pt = np.zeros((2, 64, 104), f32)   # [pair][d][sq 0:40 | sk 64:104]
    wq_ft = np.zeros((2, 64, 104), f32)
    cblk = np.zeros((2, 104, 128), f32)   # [pair][skq-row][(b, j)]
    for p in range(NPAIR):
        for b in range(2):
            bh = core * BHC + 2 * p + b
            h = bh % HEADS
            r0 = 32 * b
            wq_pt[p, :, r0 : r0 + 8] = Wsq[0, h, 0:64, :]
            wq_pt[p, :, 64 + r0 : 64 + r0 + 8] = Wsk[0, h, 0:64, :]
            wq_ft[p, :, r0 : r0 + 8] = Wsq[0, h, 64:128, :]
            wq_ft[p, :, 64 + r0 : 64 + r0 + 8] = Wsk[0, h, 64:128, :]
            cq = q_pos_emb[0, h] @ Wsq[0, h, 128:192, :]  # (64, 8)
            ck = k_pos_emb[0, h] @ Wsk[0, h, 128:192, :]
            cblk[p, r0 : r0 + 8, 64 * b : 64 * b + 64] = cq.T
            cblk[p, 64 + r0 : 64 + r0 + 8, 64 * b : 64 * b + 64] = ck.T

    c64 = np.concatenate([wq_pt[0], wq_pt[1], wq_ft[0], wq_ft[1]], axis=1)
    c64 = np.concatenate([c64, c64], axis=0)  # duplicate into both halves
    c104 = np.concatenate([cblk[0], cblk[1]], axis=1)
    return {"c128": c128, "cb": cb16, "c64": c64, "c104": c104}


def _run(k, q_pos_emb, k_pos_emb, Wsq, Wsk, trace=False):
    nc = _get_program()
    in_maps = []
    for core in range(NCORES):
        cm = _host_constants(core, q_pos_emb, k_pos_emb, Wsq, Wsk)
        cm["kin"] = np.ascontiguousarray(k[core * BHC : (core + 1) * BHC])
        in_maps.append(cm)
    res = bass_utils.run_bass_kernel_spmd(
        nc,
        in_maps,
        core_ids=list(range(NCORES)),
        trace=trace,
        **(TRACE_KWARGS if trace else {}),
    )
    global LAST_RESULTS
    LAST_RESULTS = res
    out = np.empty((BH, BUCKETS, BUCKETS + 1), np.float32)
    for core, r in enumerate(res.results):
        ro = r["rout"]  # (2, 64, 2, 65) = (b, i, pair, col)
        for p in range(NPAIR):
            for b in range(2):
                out[core * BHC + 2 * p + b] = ro[b, :, p, :]
    return out, res


def kernel(**inputs):
    k = np.asarray(inputs["k"], np.float32)
    q_pos_emb = np.asarray(inputs["q_pos_emb"], np.float32)
    k_pos_emb = np.asarray(inputs["k_pos_emb"], np.float32)
    Wsq = np.asarray(inputs["Wsq"], np.float32)
    Wsk = np.asarray(inputs["Wsk"], np.float32)
    out, _ = _run(k, q_pos_emb, k_pos_emb, Wsq, Wsk, trace=TRACE)
    return out


# revision 15
# speedup vs baseline: 1.1197x; 1.0228x over previous
"""Trainium2 Bass kernel for CausalAttentionSortNet bucket-scoring.

Math (see reference): only `k` feeds the output. For each merged batch*head
slice, the cumulative-average of k is sampled at bucket starts (every 128th
row), which reduces to per-chunk sums + a strictly-triangular prefix matmul.
The rest is tiny per-bucket sort projections and a 64x65 masked softmax.

Sharding: data-parallel over the merged (batch*heads)=32 axis across 8 cores,
4 slices per core as 2 pairs; partition=(slice_in_pair, chunk), free=(row, dim)
so every partition's k data is one contiguous 32KB HBM run. Both pairs of each
row-group share one SBUF tile so each fold is a single batched instruction.

`q` (half of all input bytes) is never read by the reference computation, so
it is not even transferred to the device.

Per row-group sub-tile: contiguous pairwise fold chains on DVE (unit-stride
fp32 tensor_tensor), with GpSimd taking a d-column share of the first level
in parallel. The folded row 0 feeds the PE prefix matmul (PT). F (row 0 of
chunk) is read straight out of sub-tile 0 before the folds clobber it - no
separate gather DMA. The 64x65 softmax keeps the zero-logit column explicit
in PSUM so the tail is one exp + one scale-mask per pair. Junk matmuls gated
on the stream keep the PE's HAM clock at 8/8 for the tail matmuls.
"""

from contextlib import ExitStack

import numpy as np

import concourse.bacc as bacc
import concourse.mybir as mybir
import concourse.tile as tile
from concourse import bass_utils

# Problem constants (hardcoded per contract; kernel.py must be self-contained).
B, HEADS, BUCKETS, DIM, DIM_SORT, T = 4, 8, 64, 64, 8, 8192
BH = B * HEADS            # 32 merged batch*head slices
NCORES = 8
BHC = BH // NCORES        # 4 slices per core
NPAIR = BHC // 2          # 2 pairs per core
CHUNK = T // BUCKETS      # 128 rows per bucket
NEG = -1.0e30             # softmax mask value (underflows exp to exactly 0)
FP = mybir.dt.float32
BF = mybir.dt.bfloat16

# rows-per-sub-tile (per pair); ascending-then-descending so folds start early
# and the tail tile is tiny. Sum = 128. All folds run on DVE: GpSimd
# tensor_tensor is ~3x slower AND degrades concurrent DVE ops ~4x (measured),
# so handing it any fold share is a net loss.
ROWS = (12, 24, 32, 32, 16, 8, 4)

TRACE = False  # set by test.py for profiling runs
TRACE_KWARGS = {}  # extra run_bass_kernel_spmd kwargs for profiling runs
LAST_RESULTS = None  # BassKernelResults of the most recent run

_PROG_CACHE = {}


def _chain(rs):
    """Pairwise fold schedule for rs rows.

    Returns (ops, final) where ops are in-place (dst_lo, dst_hi, src_lo,
    src_hi) folds and final = (row_a, row_b) whose sum is the column total
    (written to the separate contiguous par tile so it can be a matmul lhsT).
    """
    ops = []
    leftovers = []
    n = rs
    while n > 3:
        h = n // 2
        ops.append((0, h, h, 2 * h))
        if n % 2:
            leftovers.append(n - 1)
        n = h
    if n == 3:
        ops.append((1, 2, 2, 3))
        n = 2
    assert n == 2, rs
    for r in leftovers:
        ops.append((1, 2, r, r + 1))
    return ops, (0, 1)


def _build_program(enable_asserts=False):
    assert sum(ROWS) == CHUNK, (ROWS, CHUNK)
    nsub = len(ROWS)

    nc = bacc.Bacc(
        "TRN2",
        target_bir_lowering=False,
        debug=False,
        enable_asserts=enable_asserts,
        num_devices=NCORES,
    )

    def din(name, shape, dt=FP):
        return nc.dram_tensor(name, shape, dt, kind="ExternalInput").ap()

    kin = din("kin", (BHC, T, DIM))
    # packed fp32 constants (one DMA):
    # c128: [lmat_s | idents | ident | amask65 | mmask65]
    c128 = din("c128", (128, 128 * 3 + 2 * (BUCKETS + 1)))
    # cb: bf16 identity for PE warm-up matmuls
    cb = din("cb", (128, 128), BF)
    # c64: [wqk_pt_p0 | wqk_pt_p1 | wqk_ft_p0 | wqk_ft_p1], each (., 104):
    #   cols 0:40 = sort-q weights, cols 64:104 = sort-k weights; the 64 weight
    #   rows are duplicated into both partition halves so each pair's matmul
    #   can read them at the same base partition as its PT/FT slice
    c64 = din("c64", (128, 4 * 104))
    # c104: per pair a (104, 128) block: rows 0:40 = cqt, 64:104 = ckt - added
    # into the SKQ psum group by an identity-weighted matmul
    c104 = din("c104", (104, 2 * 128))
    # out layout (b, i, pair, col): 520B contiguous per (b, i) partition
    rout = nc.dram_tensor(
        "rout", (2, BUCKETS, NPAIR, BUCKETS + 1), FP, kind="ExternalOutput"
    ).ap()

    Exp = mybir.ActivationFunctionType.Exp
    MULT = mybir.AluOpType.mult
    X = mybir.AxisListType.X

    with tile.TileContext(nc) as tc:
        with ExitStack() as ctx:
            singles = ctx.enter_context(tc.tile_pool(name="singles", bufs=1))
            kpool = ctx.enter_context(tc.tile_pool(name="kpool", bufs=1))
            small = ctx.enter_context(tc.tile_pool(name="small", bufs=2))
            pp = ctx.enter_context(tc.tile_pool(name="pp", bufs=1, space="PSUM"))

            # ---- constant DMAs on the scalar queue (bulk owns sync)
            c128_sb = singles.tile([128, 128 * 3 + 2 * (BUCKETS + 1)], FP, tag="c128")
            nc.scalar.dma_start(c128_sb[:], c128)
            cb_sb = singles.tile([128, 128], BF, tag="cb")
            nc.scalar.dma_start(cb_sb[:], cb)
            c64_sb = singles.tile([128, 4 * 104], FP, tag="c64")
            nc.scalar.dma_start(c64_sb[:], c64)
            c104_sb = singles.tile([104, 2 * 128], FP, tag="c104")
            nc.scalar.dma_start(c104_sb[:], c104)

            # ---- bulk k DMAs on the sync queue, pairs interleaved into one
            # shared tile per sub-tile so folds batch both pairs
            ksrcs = [
                kin[2 * p : 2 * p + 2].rearrange("b (c r) d -> (b c) r d", r=CHUNK)
                for p in range(NPAIR)
            ]
            kts = []
            r0 = 0
            for s, rs in enumerate(ROWS):
                kt = kpool.tile([128, NPAIR, rs, DIM], FP, tag=f"kt{s}")
                for p in range(NPAIR):
                    nc.sync.dma_start(kt[:, p], ksrcs[p][:, r0 : r0 + rs, :])
                kts.append(kt)
                r0 += rs

            lmat_s = c128_sb[:, 0:128]
            idents = c128_sb[:, 128:256]
            ident = c128_sb[:, 256:384]
            amask = c128_sb[:, 384 : 384 + 65]
            mmask = c128_sb[:, 449 : 449 + 65]

            # ---- PSUM tiles
            PT_ps = pp.tile([128, 128], FP, tag="PT")
            FT_ps = pp.tile([128, 128], FP, tag="FT")
            SKQs = [
                pp.tile([104, 128], FP, tag=f"SKQ{p}", name=f"SKQ{p}")
                for p in range(NPAIR)
            ]
            R_ps = pp.tile([128, NPAIR, BUCKETS + 1], FP, tag="R")
            dummy = pp.tile([128, 128], FP, tag="dummy")

            # ---- early PE work (consts-gated): R mask seeds + SKQ const seeds
            # one start=True per PSUM bank: start clears has_written bank-wide,
            # so only the first seed may carry it
            for p in range(NPAIR):
                nc.tensor.matmul(
                    R_ps[:, p, :], lhsT=ident, rhs=amask,
                    start=p == 0, stop=False, skip_group_check=True,
                )
            for p in range(NPAIR):
                nc.tensor.matmul(
                    SKQs[p][:], lhsT=ident[0:104, 0:104],
                    rhs=c104_sb[:, 128 * p : 128 * p + 128],
                    start=True, stop=False, skip_group_check=True,
                )

            # ---- F path: copy row 0 of sub-tile 0 to a contiguous tile
            # (matmul lhsT needs one flat free dim) before folds clobber it
            kt0 = kts[0]
            F_sb = small.tile([128, NPAIR, DIM], FP, tag="F")
            nc.vector.tensor_copy(F_sb[:], kt0[:, :, 0, :])
            nc.tensor.matmul(
                PT_ps[:], lhsT=F_sb[:], rhs=idents,
                start=True, stop=False, skip_group_check=True,
            )
            nc.tensor.matmul(
                FT_ps[:], lhsT=F_sb[:], rhs=ident, start=True, stop=True
            )
            FT_sb = small.tile([128, 128], FP, tag="FTs")
            nc.scalar.copy(FT_sb[:], FT_ps[:])
            for p in range(NPAIR):
                prow = slice(64 * p, 64 * p + 64)
                nc.tensor.matmul(
                    SKQs[p][:],
                    lhsT=c64_sb[prow, 208 + 104 * p : 208 + 104 * p + 104],
                    rhs=FT_sb[prow, :],
                    start=False, stop=False, skip_group_check=True,
                )

            # ---- per-sub-tile: batched fold chain then PT accumulation.
            # Junk matmuls gated on each pair-DMA keep the PE HAM clock warm.
            for s, rs in enumerate(ROWS):
                kt = kts[s]
                if s >= 1 and s < nsub - 1:
                    for p in range(NPAIR):
                        nc.tensor.matmul(
                            dummy[0:64, :], lhsT=kt[:, p, rs - 1, :], rhs=ident,
                            start=True, stop=True, skip_group_check=True,
                        )
                ops, (fa, fb) = _chain(rs)
                for dlo, dhi, slo, shi in ops:
                    nc.vector.tensor_add(
                        kt[:, :, dlo:dhi, :],
                        kt[:, :, dlo:dhi, :],
                        kt[:, :, slo:shi, :],
                    )
                par = kpool.tile([128, NPAIR, DIM], FP, tag=f"par{s}", name=f"par{s}")
                nc.vector.tensor_add(par[:], kt[:, :, fa, :], kt[:, :, fb, :])
                nc.tensor.matmul(
                    PT_ps[:], lhsT=par[:], rhs=lmat_s,
                    start=False, stop=s == nsub - 1, skip_group_check=True,
                )
                if s == 0:
                    # warm-up burst: >=3.4us of sustained PE activity flips
                    # the HAM clock gate to 8/8 for the rest of the kernel
                    for _ in range(40):
                        nc.tensor.matmul(
                            dummy[:], lhsT=cb_sb[:], rhs=cb_sb[:],
                            start=True, stop=True, skip_group_check=True,
                        )

            # ---- tail: PT -> SBUF -> sort projections -> R -> softmax -> out
            PT_sb = small.tile([128, 128], FP, tag="PTs")
            nc.scalar.copy(PT_sb[:], PT_ps[:])
            for p in range(NPAIR):
                prow = slice(64 * p, 64 * p + 64)
                nc.tensor.matmul(
                    SKQs[p][:],
                    lhsT=c64_sb[prow, 104 * p : 104 * p + 104],
                    rhs=PT_sb[prow, :],
                    start=False, stop=True, skip_group_check=True,
                )
            SQs = []
            RKs = []
            for p in range(NPAIR):
                sq_sb = small.tile([40, 128], FP, tag=f"SQ{p}")
                nc.scalar.copy(sq_sb[:], SKQs[p][0:40, :])
                rk_sb = small.tile([40, 128], FP, tag=f"RK{p}")
                nc.vector.tensor_copy(rk_sb[:], SKQs[p][64:104, :])
                SQs.append(sq_sb)
                RKs.append(rk_sb)
            for p in range(NPAIR):
                nc.tensor.matmul(
                    R_ps[0:64, p, 1:],
                    lhsT=SQs[p][0:8, 0:64],
                    rhs=RKs[p][0:8, 0:64],
                    start=False, stop=False, skip_group_check=True,
                )
                nc.tensor.matmul(
                    R_ps[64:128, p, 1:],
                    lhsT=SQs[p][32:40, 64:128],
                    rhs=RKs[p][32:40, 64:128],
                    start=False, stop=p == NPAIR - 1, skip_group_check=True,
                )

            # masked softmax over 65 logits (zero-logit col 0 is explicit in
            # PSUM from the seed); pair-staggered so pair 0's output DMA is
            # in flight while pair 1 is still in softmax
            mx = small.tile([128, NPAIR], FP, tag="mx")
            nc.vector.reduce_max(mx[:], R_ps[:], axis=X)
            negm = small.tile([128, NPAIR], FP, tag="negm")
            nc.scalar.mul(negm[:], mx[:], -1.0)
            e_sb = small.tile([128, NPAIR, BUCKETS + 1], FP, tag="e")
            s1 = small.tile([128, NPAIR], FP, tag="s1")
            rin = small.tile([128, NPAIR], FP, tag="rin")
            outt = small.tile([128, NPAIR, BUCKETS + 1], FP, tag="outt")
            for p in range(NPAIR):
                nc.scalar.activation(
                    e_sb[:, p, :], R_ps[:, p, :], Exp,
                    bias=negm[:, p : p + 1], scale=1.0,
                )
                nc.vector.reduce_sum(s1[:, p : p + 1], e_sb[:, p, :], axis=X)
                nc.vector.reciprocal(rin[:, p : p + 1], s1[:, p : p + 1])
                # outt = (e * 1/den) * tril-mask, fused
                nc.vector.scalar_tensor_tensor(
                    outt[:, p, :],
                    e_sb[:, p, :],
                    rin[:, p : p + 1],
                    mmask,
                    op0=MULT,
                    op1=MULT,
                )
                dst = rout[:, :, p, :].rearrange("b i c -> (b i) c")
                if p == 0:
                    nc.scalar.dma_start(dst, outt[:, p, :])
                else:
                    nc.sync.dma_start(dst, outt[:, p, :])

    nc.compile()
    return nc


def _get_program(enable_asserts=False):
    key = enable_asserts
    if key not in _PROG_CACHE:
        _PROG_CACHE[key] = _build_program(enable_asserts=enable_asserts)
    return _PROG_CACHE[key]


def _host_constants(core, q_pos_emb, k_pos_emb, Wsq, Wsk):
    """Tiny per-core packed constant tensors."""
    f32 = np.float32
    j = np.arange(64, dtype=np.float64)
    s = (1.0 / (CHUNK * j + 1.0)).astype(f32)  # per-bucket cumavg scale

    tri = np.triu(np.ones((64, 64), f32), k=1)  # [c, j] = 1 iff c < j
    tri_s = tri * s[None, :]
    lmat_s = np.zeros((128, 128), f32)
    lmat_s[0:64, 0:64] = tri_s
    lmat_s[64:128, 64:128] = tri_s
    idents = np.zeros((128, 128), f32)
    idents[np.arange(128), np.arange(128)] = np.concatenate([s, s])
    ident = np.eye(128, dtype=f32)

    q = np.arange(64)[:, None]
    jc = np.arange(65)[None, :]
    am = np.where(jc > q, NEG, 0.0).astype(f32)   # softmax additive mask, col0 free
    mm = (jc < q).astype(f32)                     # output tril(-1) mask incl col0
    amask = np.concatenate([am, am], axis=0)      # (128, 65) both b blocks
    mmask = np.concatenate([mm, mm], axis=0)

    c128 = np.concatenate([lmat_s, idents, ident, amask, mmask], axis=1)

    import ml_dtypes

    cb16 = np.eye(128, dtype=ml_dtypes.bfloat16)

    wq_pt = np.zeros((2, 64, 104), f32)   # [pair][d][sq 0:40 | sk 64:104]
    wq_ft = np.zeros((2, 64, 104), f32)
    cblk = np.zeros((2, 104, 128), f32)   # [pair][skq-row][(b, j)]
    for p in range(NPAIR):
        for b in range(2):
            bh = core * BHC + 2 * p + b
            h = bh % HEADS
            r0 = 32 * b
            wq_pt[p, :, r0 : r0 + 8] = Wsq[0, h, 0:64, :]
            wq_pt[p, :, 64 + r0 : 64 + r0 + 8] = Wsk[0, h, 0:64, :]
            wq_ft[p, :, r0 : r0 + 8] = Wsq[0, h, 64:128, :]
            wq_ft[p, :, 64 + r0 : 64 + r0 + 8] = Wsk[0, h, 64:128, :]
            cq = q_pos_emb[0, h] @ Wsq[0, h, 128:192, :]  # (64, 8)
            ck = k_pos_emb[0, h] @ Wsk[0, h, 128:192, :]
            cblk[p, r0 : r0 + 8, 64 * b : 64 * b + 64] = cq.T
            cblk[p, 64 + r0 : 64 + r0 + 8, 64 * b : 64 * b + 64] = ck.T

    c64 = np.concatenate([wq_pt[0], wq_pt[1], wq_ft[0], wq_ft[1]], axis=1)
    c64 = np.concatenate([c64, c64], axis=0)  # duplicate into both halves
    c104 = np.concatenate([cblk[0], cblk[1]], axis=1)
    return {"c128": c128, "cb": cb16, "c64": c64, "c104": c104}


def _run(k, q_pos_emb, k_pos_emb, Wsq, Wsk, trace=False):
    nc = _get_program()
    in_maps = []
    for core in range(NCORES):
        cm = _host_constants(core, q_pos_emb, k_pos_emb, Wsq, Wsk)
        cm["kin"] = np.ascontiguousarray(k[core * BHC : (core + 1) * BHC])
        in_maps.append(cm)
    res = bass_utils.run_bass_kernel_spmd(
        nc,
        in_maps,
        core_ids=list(range(NCORES)),
        trace=trace,
        **(TRACE_KWARGS if trace else {}),
    )
    global LAST_RESULTS
    LAST_RESULTS = res
    out = np.empty((BH, BUCKETS, BUCKETS + 1), np.float32)
    for core, r in enumerate(res.results):
        ro = r["rout"]  # (2, 64, 2, 65) = (b, i, pair, col)
        for p in range(NPAIR):
            for b in range(2):
                out[core * BHC + 2 * p + b] = ro[b, :, p, :]
    return out, res


def kernel(**inputs):
    k = np.asarray(inputs["k"], np.float32)
    q_pos_emb = np.asarray(inputs["q_pos_emb"], np.float32)
    k_pos_emb = np.asarray(inputs["k_pos_emb"], np.float32)
    Wsq = np.asarray(inputs["Wsq"], np.float32)
    Wsk = np.asarray(inputs["Wsk"], np.float32)
    out, _ = _run(k, q_pos_emb, k_pos_emb, Wsq, Wsk, trace=TRACE)
    return out
